# revision 22
# baseline (speedup 1.0000x reference)
import numpy as np
import ml_dtypes

BF16 = ml_dtypes.bfloat16

V = 32000
D = 1024
H = 16
L = 6
T = 768
KV = 256
B = 2
S = 1024
DH = 64
PHYS_TOK = 5
NC_TOTAL = 8
GP = 4           # cores per tensor-parallel group (one sample per group)
RL = 256         # local qkv rows per core (4 heads x 64)
FL = 1024        # local ff dim per core
VL = V // GP     # local vocab cols per core (8000)
NH_LOC = 4       # heads per core

_prog = None

import os  # noqa: E402
_NL = int(os.environ.get("KNL_LAYERS", str(L)))
_USE_COLL = os.environ.get("KNL_COLL", "1") == "1"
_DO_HEAD = os.environ.get("KNL_HEAD", "1") == "1"
_STAGE = int(os.environ.get("KNL_STAGE", "9"))
_ATT = int(os.environ.get("KNL_ATT", "3"))


def _build_program():
    import concourse.bass as bass
    import concourse.tile as tile
    from concourse import bacc, mybir

    f32 = mybir.dt.float32
    bf16 = mybir.dt.bfloat16
    AF = mybir.ActivationFunctionType
    ALU = mybir.AluOpType
    AX = mybir.AxisListType

    nc = bacc.Bacc("TRN2", target_bir_lowering=False, debug=False)

    x0_d = nc.dram_tensor("x0", [128, 8, D], f32, kind="ExternalInput")
    mask_d = nc.dram_tensor("maskT", [128, 8, S], bf16, kind="ExternalInput")
    id_d = nc.dram_tensor("ident", [128, 128], bf16, kind="ExternalInput")
    wq_d = nc.dram_tensor("wq", [L, 128, 8, RL], bf16, kind="ExternalInput")
    wk_d = nc.dram_tensor("wk", [L, 128, 8, RL], bf16, kind="ExternalInput")
    wv_d = nc.dram_tensor("wv", [L, 128, 8, 264], bf16, kind="ExternalInput")
    wo_d = nc.dram_tensor("wo", [L, 128, 2, D], bf16, kind="ExternalInput")
    w1_d = nc.dram_tensor("w1", [L, 128, 8, FL], bf16, kind="ExternalInput")
    w2_d = nc.dram_tensor("w2", [L, 128, 8, D], bf16, kind="ExternalInput")
    wte_d = nc.dram_tensor("wteT", [128, 8, VL], bf16, kind="ExternalInput")
    out_d = nc.dram_tensor("logits", [S, VL], f32, kind="ExternalOutput")

    groups = [[0, 1, 2, 3], [4, 5, 6, 7]]

    with tile.TileContext(nc) as tc:
        from contextlib import ExitStack
        with ExitStack() as ctx:
            pers = ctx.enter_context(tc.tile_pool(name="pers", bufs=1))
            tpool = ctx.enter_context(tc.tile_pool(name="tp", bufs=1))
            cpool = ctx.enter_context(tc.tile_pool(name="coll", bufs=2))
            ppool = ctx.enter_context(tc.tile_pool(name="pbuf", bufs=1))
            rpool = ctx.enter_context(tc.tile_pool(name="rec", bufs=2))
            psA = ctx.enter_context(
                tc.tile_pool(name="psA", bufs=2, space=bass.MemorySpace.PSUM))
            psT = ctx.enter_context(
                tc.tile_pool(name="psT", bufs=2, space=bass.MemorySpace.PSUM))
            psU = ctx.enter_context(
                tc.tile_pool(name="psU", bufs=2, space=bass.MemorySpace.PSUM))
            dram = ctx.enter_context(
                tc.tile_pool(name="dram", bufs=2, space="DRAM"))

            h = pers.tile([128, 8, D], f32, tag="h")
            maskT = pers.tile([128, 8, S], bf16, tag="mask")
            ident = pers.tile([128, 128], bf16, tag="ident")
            # QK: partitions 0:64 hold dh; free dims (8, S): cols 0..3 = Q heads, 4..7 = K heads
            QK = pers.tile([64, 8, S], bf16, tag="QK")
            Vt = pers.tile([128, 8, 264], bf16, tag="Vt")
            aOtm = pers.tile([128, 8, 2, 128], bf16, tag="aOtm")
            aOf = pers.tile([128, 2, S], bf16, tag="aOf")
            gf = pers.tile([128, 8, FL], bf16, tag="gf")
            xf = pers.tile([128, 8, S], bf16, tag="xf")
            sums = pers.tile([128, 8], f32, tag="sums")
            ssq = pers.tile([128, 8], f32, tag="ssq")
            mean = pers.tile([128, 8], f32, tag="mean")
            var = pers.tile([128, 8], f32, tag="var")
            m2 = pers.tile([128, 8], f32, tag="m2")
            std = pers.tile([128, 8], f32, tag="std")
            rstd = pers.tile([128, 8], f32, tag="rstd")
            eps = pers.tile([128, 1], f32, tag="eps")
            nc.vector.memset(eps[:], 1e-5)

            nc.sync.dma_start(h[:], x0_d[:])
            nc.gpsimd.dma_start(maskT[:], mask_d[:])
            nc.sync.dma_start(ident[:], id_d[:])

            def ln_stats_finish():
                nc.vector.tensor_scalar(mean[:], sums[:], 1.0 / D, None,
                                        op0=ALU.mult)
                nc.vector.tensor_scalar(var[:], ssq[:], 1.0 / D, None,
                                        op0=ALU.mult)
                nc.scalar.activation(m2[:], mean[:], AF.Square)
                nc.vector.tensor_sub(var[:], var[:], m2[:])
                nc.scalar.activation(std[:], var[:], AF.Sqrt, bias=eps[:])
                nc.vector.reciprocal(rstd[:], std[:])

            def ln_sumsq(lname):
                for i in range(8):
                    sq = ppool.tile([128, D], f32, tag="P",
                                    name=f"sq_{lname}_{i}")
                    nc.scalar.activation(sq[:], h[:, i, :], AF.Square,
                                         accum_out=ssq[:, i:i + 1])

            def ln_apply_transpose(lname):
                # (h - mean) * rstd -> bf16, then PE-transpose into xf
                for i in range(8):
                    xn = tpool.tile([128, D], bf16, tag="xn", bufs=2,
                                    name=f"xn_{lname}_{i}")
                    nc.vector.tensor_scalar(xn[:], h[:, i, :],
                                            mean[:, i:i + 1], rstd[:, i:i + 1],
                                            op0=ALU.subtract, op1=ALU.mult)
                    for j in range(8):
                        pt = psT.tile([128, 128], bf16, tag="T",
                                      name=f"pt_{lname}_{i}_{j}")
                        nc.tensor.transpose(pt[:], xn[:, 128 * j:128 * (j + 1)],
                                            ident[:])
                        eng = nc.scalar if (j % 2 == 0) else nc.vector
                        if j % 2 == 0:
                            eng.copy(xf[:, j, 128 * i:128 * (i + 1)], pt[:])
                        else:
                            eng.tensor_copy(xf[:, j, 128 * i:128 * (i + 1)],
                                            pt[:])

            for l in range(_NL):
                wq = tpool.tile([128, 8, RL], bf16, tag="wq", bufs=1,
                                name=f"wq{l}")
                wk = tpool.tile([128, 8, RL], bf16, tag="wk", bufs=1,
                                name=f"wk{l}")
                wv = tpool.tile([128, 8, 264], bf16, tag="wv", bufs=1,
                                name=f"wv{l}")
                wo = tpool.tile([128, 2, D], bf16, tag="wo", bufs=1,
                                name=f"wo{l}")
                w1 = tpool.tile([128, 8, FL], bf16, tag="w1", bufs=1,
                                name=f"w1{l}")
                w2 = tpool.tile([128, 8, D], bf16, tag="w2", bufs=1,
                                name=f"w2{l}")
                nc.sync.dma_start(wq[:], wq_d[l])
                nc.sync.dma_start(wk[:], wk_d[l])
                nc.sync.dma_start(wv[:], wv_d[l])
                nc.sync.dma_start(wo[:], wo_d[l])
                nc.sync.dma_start(w1[:], w1_d[l])
                nc.sync.dma_start(w2[:], w2_d[l])

                # ---------- LN1 ----------
                if l == 0:
                    for i in range(8):
                        nc.vector.tensor_reduce(sums[:, i:i + 1], h[:, i, :],
                                                axis=AX.X, op=ALU.add)
                ln_sumsq(f"l{l}a")
                ln_stats_finish()
                ln_apply_transpose(f"l{l}a")

                # ---------- QKV ----------
                for hd in range(NH_LOC if _STAGE >= 2 else 0):
                    for kq in range(2):  # 0 = Q, 1 = K
                        w = wq if kq == 0 else wk
                        for ch in range(2):
                            ps = psA.tile([128, 512], f32, tag="A",
                                          name=f"psqk{l}_{hd}_{kq}_{ch}")
                            for j in range(8):
                                nc.tensor.matmul(
                                    ps[0:64, :],
                                    w[:, j, 64 * hd:64 * (hd + 1)],
                                    xf[:, j, 512 * ch:512 * (ch + 1)],
                                    start=(j == 0), stop=(j == 7))
                            nc.scalar.copy(
                                QK[:, 4 * kq + hd, 512 * ch:512 * (ch + 1)],
                                ps[0:64, :])
                for ti in range(8 if _STAGE >= 2 else 0):
                    ps = psA.tile([128, 512], f32, tag="A", name=f"psv{l}_{ti}")
                    for j in range(8):
                        nc.tensor.matmul(ps[:, 0:264],
                                         xf[:, j, 128 * ti:128 * (ti + 1)],
                                         wv[:, j, :],
                                         start=(j == 0), stop=(j == 7))
                    nc.scalar.copy(Vt[:, ti, :], ps[:, 0:264])
                for hd in range(NH_LOC if _STAGE >= 2 else 0):
                    nc.vector.memset(Vt[:, :, 66 * hd + 64:66 * hd + 65], 1.0)

                # ---------- attention per head ----------
                for hd in range(NH_LOC if _STAGE >= 3 else 0):
                    P = ppool.tile([128, 8, S], bf16, tag="P",
                                   name=f"P{l}_{hd}")
                    for kt in range(8):
                        qlo = 0 if kt < 3 else 128 * kt
                        if qlo < 512:
                            chunks = [(qlo, 512 - qlo), (512, 512)]
                        else:
                            chunks = [(qlo, 1024 - qlo)]
                        for (q0, qn) in chunks:
                            ps = psA.tile([128, 512], f32, tag="A",
                                          name=f"pss{l}_{hd}_{kt}_{q0}")
                            nc.tensor.matmul(
                                ps[:, 0:qn],
                                QK[:, 4 + hd, 128 * kt:128 * (kt + 1)],
                                QK[:, hd, q0:q0 + qn],
                                start=True, stop=True)
                            nc.scalar.activation(P[:, kt, q0:q0 + qn],
                                                 ps[:, 0:qn], AF.Exp,
                                                 scale=0.125)
                            nc.vector.tensor_mul(P[:, kt, q0:q0 + qn],
                                                 P[:, kt, q0:q0 + qn],
                                                 maskT[:, kt, q0:q0 + qn])
                    for qt in range(8 if _ATT >= 2 else 0):
                        upper = max(qt, 2)
                        ps = psU.tile([128, 128], f32, tag="U",
                                      name=f"psu{l}_{hd}_{qt}")
                        for kt in range(upper + 1):
                            nc.tensor.matmul(
                                ps[:, 0:65],
                                P[:, kt, 128 * qt:128 * (qt + 1)],
                                Vt[:, kt, 66 * hd:66 * hd + 65],
                                start=(kt == 0), stop=(kt == upper))
                        off = 64 * (hd % 2)
                        if _ATT >= 3:
                            rec = rpool.tile([128, 1], f32, tag="rec",
                                             name=f"rec{l}_{hd}_{qt}")
                            nc.vector.reciprocal(rec[:], ps[:, 64:65])
                            nc.vector.tensor_scalar(
                                aOtm[:, qt, hd // 2, off:off + 64],
                                ps[:, 0:64], rec[:], None, op0=ALU.mult)
                        else:
                            nc.scalar.copy(
                                aOtm[:, qt, hd // 2, off:off + 64],
                                ps[:, 0:64])

                # transpose attn output to feature-major
                for qt in range(8 if _STAGE >= 4 else 0):
                    for hp in range(2):
                        pt = psT.tile([128, 128], bf16, tag="T",
                                      name=f"pta{l}_{qt}_{hp}")
                        nc.tensor.transpose(pt[:], aOtm[:, qt, hp, :],
                                            ident[:])
                        nc.vector.tensor_copy(aOf[:, hp, 128 * qt:128 * (qt + 1)],
                                              pt[:])

                # ---------- out-proj + AllReduce ----------
                cin = dram.tile([128, 8, D], f32, tag="cin", name=f"cin_a{l}")
                cout = dram.tile([128, 8, D], f32, tag="cout", name=f"cout_a{l}")
                for tt in range(8 if _STAGE >= 4 else 0):
                    for mc in range(2):
                        ps = psA.tile([128, 512], f32, tag="A",
                                      name=f"pso{l}_{tt}_{mc}")
                        for rp in range(2):
                            nc.tensor.matmul(
                                ps[:],
                                aOf[:, rp, 128 * tt:128 * (tt + 1)],
                                wo[:, rp, 512 * mc:512 * (mc + 1)],
                                start=(rp == 0), stop=(rp == 1))
                        st = cpool.tile([128, 512], f32, tag="c",
                                        name=f"sto{l}_{tt}_{mc}")
                        nc.scalar.copy(st[:], ps[:])
                        nc.gpsimd.dma_start(
                            cin[:, tt, 512 * mc:512 * (mc + 1)], st[:])
                if _USE_COLL:
                    nc.gpsimd.collective_compute(
                        "AllReduce", ALU.add, replica_groups=groups,
                        ins=[cin.opt()], outs=[cout.opt()])
                else:
                    nc.gpsimd.dma_start(cout[:], cin[:])
                for i in range(8 if _STAGE >= 5 else 0):
                    red = cpool.tile([128, D], f32, tag="c", name=f"reda{l}_{i}")
                    nc.gpsimd.dma_start(red[:], cout[:, i, :])
                    nc.vector.scalar_tensor_tensor(
                        out=h[:, i, :], in0=h[:, i, :], scalar=1.0,
                        in1=red[:], op0=ALU.mult, op1=ALU.add,
                        accum_out=sums[:, i:i + 1])

                # ---------- LN2 + FFN ----------
                if _STAGE < 6:
                    continue
                ln_sumsq(f"l{l}b")
                ln_stats_finish()
                ln_apply_transpose(f"l{l}b")

                for fi in range(8):
                    for ch in range(2):
                        ps = psA.tile([128, 512], f32, tag="A",
                                      name=f"psf1{l}_{fi}_{ch}")
                        for j in range(8):
                            nc.tensor.matmul(
                                ps[:],
                                w1[:, j, 128 * fi:128 * (fi + 1)],
                                xf[:, j, 512 * ch:512 * (ch + 1)],
                                start=(j == 0), stop=(j == 7))
                        nc.scalar.activation(gf[:, fi, 512 * ch:512 * (ch + 1)],
                                             ps[:], AF.Gelu)

                cin2 = dram.tile([128, 8, D], f32, tag="cin", name=f"cin_f{l}")
                cout2 = dram.tile([128, 8, D], f32, tag="cout", name=f"cout_f{l}")
                for tt in range(8):
                    for mc in range(2):
                        ps = psA.tile([128, 512], f32, tag="A",
                                      name=f"psf2{l}_{tt}_{mc}")
                        for fj in range(8):
                            nc.tensor.matmul(
                                ps[:],
                                gf[:, fj, 128 * tt:128 * (tt + 1)],
                                w2[:, fj, 512 * mc:512 * (mc + 1)],
                                start=(fj == 0), stop=(fj == 7))
                        st = cpool.tile([128, 512], f32, tag="c",
                                        name=f"stf{l}_{tt}_{mc}")
                        nc.scalar.copy(st[:], ps[:])
                        nc.gpsimd.dma_start(
                            cin2[:, tt, 512 * mc:512 * (mc + 1)], st[:])
                if _USE_COLL:
                    nc.gpsimd.collective_compute(
                        "AllReduce", ALU.add, replica_groups=groups,
                        ins=[cin2.opt()], outs=[cout2.opt()])
                else:
                    nc.gpsimd.dma_start(cout2[:], cin2[:])
                for i in range(8):
                    red = cpool.tile([128, D], f32, tag="c", name=f"redf{l}_{i}")
                    nc.gpsimd.dma_start(red[:], cout2[:, i, :])
                    nc.vector.scalar_tensor_tensor(
                        out=h[:, i, :], in0=h[:, i, :], scalar=1.0,
                        in1=red[:], op0=ALU.mult, op1=ALU.add,
                        accum_out=sums[:, i:i + 1])

            # ---------- final LN + LM head ----------
            ln_sumsq("lnf")
            ln_stats_finish()
            ln_apply_transpose("lnf")

            n_vc = (VL + 511) // 512
            for vc in range(n_vc if _DO_HEAD else 1):
                w = min(512, VL - 512 * vc)
                wt = ppool.tile([128, 8, 512], bf16, tag="P", name=f"wte{vc}")
                nc.sync.dma_start(wt[:, :, 0:w],
                                  wte_d[:, :, 512 * vc:512 * vc + w])
                for tt in range(8):
                    ps = psA.tile([128, 512], f32, tag="A",
                                  name=f"pslm{vc}_{tt}")
                    for j in range(8):
                        nc.tensor.matmul(ps[:, 0:w],
                                         xf[:, j, 128 * tt:128 * (tt + 1)],
                                         wt[:, j, 0:w],
                                         start=(j == 0), stop=(j == 7))
                    st = cpool.tile([128, 512], f32, tag="c",
                                    name=f"stlm{vc}_{tt}")
                    nc.scalar.copy(st[:, 0:w], ps[:, 0:w])
                    nc.gpsimd.dma_start(
                        out_d[128 * tt:128 * (tt + 1),
                              512 * vc:512 * vc + w], st[:, 0:w])

    nc.compile()
    return nc


def _tile_pm(a):
    # [R, C] with R = 8*128 -> [128, 8, C] partition-major tiling
    R = a.shape[0]
    rest = a.shape[1:]
    return np.ascontiguousarray(
        a.reshape((8, 128) + rest).transpose((1, 0) + tuple(range(2, 2 + len(rest)))))


def _prep_host(inputs):
    idx = np.asarray(inputs["idx"])
    vis = np.asarray(inputs["vision_embeds"], np.float32)
    wte = np.asarray(inputs["wte"], np.float32)
    wpe = np.asarray(inputs["wpe"], np.float32)
    qkv_w = np.asarray(inputs["qkv_w"], np.float32)
    out_w = np.asarray(inputs["out_w"], np.float32)
    ff1_w = np.asarray(inputs["ff1_w"], np.float32)
    ff2_w = np.asarray(inputs["ff2_w"], np.float32)

    tok_emb = wte[idx]  # [B, T, D]
    eq = idx == PHYS_TOK
    pe = np.where(eq.any(axis=1), eq.argmax(axis=1) + 1, 0)  # [B]

    p = np.arange(S)
    x0 = np.empty((B, S, D), np.float32)
    maskT = np.empty((B, S, S), np.float32)
    for b in range(B):
        peb = int(pe[b])
        use_vis = (p >= peb) & (p < peb + KV)
        tok_idx = np.clip(np.where(p < peb, p, p - KV), 0, T - 1)
        vis_idx = np.clip(p - peb, 0, KV - 1)
        xb = np.where(use_vis[:, None], vis[b][vis_idx], tok_emb[b][tok_idx])
        x0[b] = xb + wpe[:S]
        q = p[:, None]
        kk = p[None, :]
        ve = peb + KV
        allowed = ((q < peb) & (kk < peb)) \
            | ((q >= peb) & (q < ve) & (kk < ve)) \
            | ((q >= ve) & (kk <= q))
        maskT[b] = allowed.T.astype(np.float32)

    in_maps = []
    for c in range(NC_TOTAL):
        g = c // GP
        lc = c % GP
        m = {}
        m["x0"] = _tile_pm(x0[g])
        m["maskT"] = _tile_pm(maskT[g]).astype(BF16)
        m["ident"] = np.eye(128, dtype=BF16)
        # Q/K/V row shards: rows 256*lc .. 256*(lc+1) of each block
        wq = qkv_w[:, 256 * lc:256 * (lc + 1), :]            # [L, 256, D]
        wk = qkv_w[:, D + 256 * lc:D + 256 * (lc + 1), :]
        wvr = qkv_w[:, 2 * D + 256 * lc:2 * D + 256 * (lc + 1), :]
        wqT = np.ascontiguousarray(wq.transpose(0, 2, 1))    # [L, D, 256]
        wkT = np.ascontiguousarray(wk.transpose(0, 2, 1))
        wv_pad = np.zeros((L, D, 4, 66), np.float32)
        wvT = wvr.transpose(0, 2, 1)                         # [L, D, 256]
        for hd in range(4):
            wv_pad[:, :, hd, 0:64] = wvT[:, :, 64 * hd:64 * (hd + 1)]
        wv_pad = wv_pad.reshape(L, D, 264)
        woT = np.ascontiguousarray(
            out_w[:, :, 256 * lc:256 * (lc + 1)].transpose(0, 2, 1))  # [L,256,D]
        w1T = np.ascontiguousarray(
            ff1_w[:, FL * lc:FL * (lc + 1), :].transpose(0, 2, 1))    # [L,D,FL]
        w2T = np.ascontiguousarray(
            ff2_w[:, :, FL * lc:FL * (lc + 1)].transpose(0, 2, 1))    # [L,FL,D]
        wteT = np.ascontiguousarray(
            wte[VL * lc:VL * (lc + 1), :].T)                          # [D, VL]

        def tile_w(a):  # [L, R, C] -> [L, 128, R//128, C]
            Lx, R, C = a.shape
            return np.ascontiguousarray(
                a.reshape(Lx, R // 128, 128, C).transpose(0, 2, 1, 3)
            ).astype(BF16)

        m["wq"] = tile_w(wqT)
        m["wk"] = tile_w(wkT)
        m["wv"] = tile_w(wv_pad)
        m["wo"] = tile_w(woT)
        m["w1"] = tile_w(w1T)
        m["w2"] = tile_w(w2T)
        m["wteT"] = _tile_pm(wteT).astype(BF16)
        in_maps.append(m)
    return in_maps


def kernel(**inputs):
    global _prog
    import time
    from concourse.bass_utils import run_bass_kernel_spmd
    if _prog is None:
        _prog = _build_program()
    in_maps = _prep_host(inputs)
    last_exc = None
    for attempt in range(4):
        try:
            res = run_bass_kernel_spmd(_prog, in_maps, list(range(NC_TOTAL)))
            break
        except Exception as e:
            # axon worker occasionally wedges; wait for recovery and retry
            last_exc = e
            time.sleep(40 * (attempt + 1))
    else:
        raise last_exc
    logits = np.empty((B, S, V), np.float32)
    for c in range(NC_TOTAL):
        g = c // GP
        lc = c % GP
        logits[g, :, VL * lc:VL * (lc + 1)] = res.results[c]["logits"]
    return logits
